# revision 20
# baseline (speedup 1.0000x reference)
"""Associative-embedding loss kernel for 8 Trainium2 NeuronCores.

Math: per image b, with tl[n,c] = pred[b,c,ty,tx] and br[n,c] = target[b,c,by,bx]
gathered at the N=128 match points:
  pull_b = sum_{n,c} (tl-br)^2 / (2N)
  s[n]   = 0.5 * sum_c (tl+br),  A[i,j] = s[i]-s[j]
  push_b = sum_{i!=j} relu(1-|A[i,j]|) / (N(N-1))
Using antisymmetry of A:  sum_{ij} relu(1-|A|) = sum|A+1| - sum|A|, and the
diagonal contributes exactly N, so push_b = (sum|A+1| - sum|A| - N)/(N(N-1)).

Strategy: data-parallel over B (8 images per core). Inputs are relaid out
channels-last on the host so each match point is one contiguous 16B gather.
The device kernel issues 16 indirect DMAs (one per image x corner, 128
descriptors each); each image's compute (channel sums, two tiny PE
transposes, one K=2 matmul building the pairwise-difference matrix, and
|A+1|/|A| accumulating reductions) is hidden under the next image's
descriptor generation. Each core returns [128, 24] partial sums that the
host folds into the two scalars.
"""

import numpy as np

B, C, H, W, N = 64, 4, 256, 256, 128
M = 8            # cores
BL = B // M      # images per core
HW = H * W

_GRAPH = None


def _build_graph():
    import concourse.bass as bass
    import concourse.bacc as bacc
    import concourse.mybir as mybir
    from concourse.tile import TileContext

    f32 = mybir.dt.float32
    i32 = mybir.dt.int32
    Alu = mybir.AluOpType
    Act = mybir.ActivationFunctionType

    nc = bacc.Bacc()
    pt_d = nc.declare_dram_parameter("pt", [1024, 1024, C], f32, isOutput=False)
    idx_d = nc.declare_dram_parameter("idx", [N, 2 * BL], i32, isOutput=False)
    ident_d = nc.declare_dram_parameter("ident", [128, 128], f32, isOutput=False)
    out_d = nc.declare_dram_parameter("out", [N, 3 * BL], f32, isOutput=True)

    with TileContext(nc) as tc:
        with (
            tc.tile_pool(name="sb", bufs=1) as pool,
            tc.tile_pool(name="w", bufs=1) as wpool,
            tc.tile_pool(name="ps", bufs=2, space="PSUM") as psum,
        ):
            idx_t = pool.tile([N, 2 * BL], i32)
            nc.sync.dma_start(out=idx_t[:], in_=idx_d[:])
            ident = pool.tile([128, 128], f32)
            nc.sync.dma_start(out=ident[:], in_=ident_d[:])

            acc = pool.tile([N, 3 * BL], f32)

            # pre-fill the transpose sources with ones (off the critical path)
            x2a = [wpool.tile([N, 2], f32, name=f"x2a{b}", tag=f"x2a{b}") for b in range(BL)]
            x2b = [wpool.tile([N, 2], f32, name=f"x2b{b}", tag=f"x2b{b}") for b in range(BL)]
            for b in range(BL):
                nc.vector.memset(x2a[b][:], 1.0)
                nc.vector.memset(x2b[b][:], 1.0)

            # idx col 2b = tl of image b, col 2b+1 = br of image b
            g = [wpool.tile([N, 2 * C], f32, name=f"g{b}", tag=f"g{b}") for b in range(BL)]
            for b in range(BL):
                for half in range(2):
                    k = 2 * b + half
                    nc.gpsimd.indirect_dma_start(
                        out=g[b][:, C * half:C * (half + 1)], out_offset=None,
                        in_=pt_d[:],
                        in_offset=bass.IndirectOffsetOnAxis(
                            ap=idx_t[:, k:k + 1], axis=1),
                    )

            for b in range(BL):
                tl = g[b][:, 0:C]
                br = g[b][:, C:2 * C]

                # pull: acc[:, 3b] = sum_c (tl-br)^2
                d = wpool.tile([N, C], f32, tag=f"d{b}")
                nc.vector.tensor_sub(d[:], tl, br)
                d2 = wpool.tile([N, C], f32, tag=f"d2{b}")
                nc.scalar.activation(
                    out=d2[:], in_=d[:], func=Act.Square,
                    accum_out=acc[:, 3 * b:3 * b + 1],
                )

                # t = sum_c (tl+br) = 2*s
                e = wpool.tile([N, C], f32, tag=f"e{b}")
                nc.vector.tensor_add(e[:], tl, br)
                t = wpool.tile([N, 1], f32, tag=f"t{b}")
                nc.vector.tensor_reduce(
                    out=t[:], in_=e[:], axis=mybir.AxisListType.X, op=Alu.add)

                # x2a cols = [0.5t, 1]; x2b cols = [1, -0.5t]
                nc.scalar.activation(out=x2a[b][:, 0:1], in_=t[:], func=Act.Copy, scale=0.5)
                nc.scalar.activation(out=x2b[b][:, 1:2], in_=t[:], func=Act.Copy, scale=-0.5)

                ta_ps = psum.tile([2, 128], f32, tag="ta")
                tb_ps = psum.tile([2, 128], f32, tag="tb")
                nc.tensor.transpose(out=ta_ps[:], in_=x2a[b][:], identity=ident[:])
                nc.tensor.transpose(out=tb_ps[:], in_=x2b[b][:], identity=ident[:])
                lt = wpool.tile([2, 128], f32, tag=f"lt{b}")
                rs = wpool.tile([2, 128], f32, tag=f"rs{b}")
                nc.vector.tensor_copy(lt[:], ta_ps[:])
                nc.vector.tensor_copy(rs[:], tb_ps[:])

                # A[i,j] = 0.5t[i] - 0.5t[j]
                a_ps = psum.tile([128, 128], f32, tag="a", bufs=4)
                nc.tensor.matmul(out=a_ps[:], lhsT=lt[:], rhs=rs[:],
                                 start=True, stop=True)

                # acc[:, 3b+1] = sum_j |A+1|, acc[:, 3b+2] = sum_j |A|
                scr = wpool.tile([128, 128], f32, tag=f"scr{b}")
                nc.scalar.activation(
                    out=scr[:], in_=a_ps[:], func=Act.Abs, bias=1.0, scale=1.0,
                    accum_out=acc[:, 3 * b + 1:3 * b + 2],
                )
                if b < BL - 1:
                    scr2 = wpool.tile([128, 128], f32, tag=f"scr2{b}")
                    nc.scalar.activation(
                        out=scr2[:], in_=a_ps[:], func=Act.Abs,
                        accum_out=acc[:, 3 * b + 2:3 * b + 3],
                    )
                else:
                    nc.vector.tensor_reduce(
                        out=acc[:, 3 * b + 2:3 * b + 3], in_=a_ps[:],
                        axis=mybir.AxisListType.X, op=Alu.add,
                        apply_absolute_value=True,
                    )

            nc.sync.dma_start(out=out_d[:, 0:3 * (BL - 2)], in_=acc[:, 0:3 * (BL - 2)])
            nc.sync.dma_start(out=out_d[:, 3 * (BL - 2):], in_=acc[:, 3 * (BL - 2):])
    nc.finalize()
    return nc


def _get_graph():
    global _GRAPH
    if _GRAPH is None:
        _GRAPH = _build_graph()
    return _GRAPH


def _make_in_maps(pred, target, match):
    pred_cl = np.ascontiguousarray(np.transpose(pred, (0, 2, 3, 1)))
    targ_cl = np.ascontiguousarray(np.transpose(target, (0, 2, 3, 1)))
    ident = np.eye(128, dtype=np.float32)
    in_maps = []
    base = (np.arange(BL, dtype=np.int64) * HW)[:, None]
    for i in range(M):
        sl = slice(i * BL, (i + 1) * BL)
        m = match[sl].astype(np.int64)
        itl = base + m[:, :, 0, 0] * W + m[:, :, 0, 1]            # [BL, N]
        ibr = BL * HW + base + m[:, :, 1, 0] * W + m[:, :, 1, 1]  # [BL, N]
        # interleave: col 2b = tl_b, col 2b+1 = br_b
        idx = np.empty((N, 2 * BL), np.int32)
        idx[:, 0::2] = itl.T
        idx[:, 1::2] = ibr.T
        pt = np.concatenate(
            [pred_cl[sl].reshape(512, 1024, C), targ_cl[sl].reshape(512, 1024, C)],
            axis=0,
        )
        in_maps.append({"pt": pt, "idx": idx, "ident": ident})
    return in_maps


def _finish(core_outs):
    pull_total = 0.0
    m_total = 0.0
    for o in core_outs:
        o = o.astype(np.float64).reshape(N, BL, 3)
        pull_total += o[:, :, 0].sum()
        m_total += (o[:, :, 1] - o[:, :, 2]).sum()
    pull_all = 0.25 * pull_total / (2 * N)
    push_all = 0.25 * (m_total - B * N) / (N * (N - 1))
    return (np.float32(pull_all), np.float32(push_all))


def kernel(pred, target, match):
    from concourse.bass_utils import run_bass_kernel_spmd

    nc = _get_graph()
    in_maps = _make_in_maps(np.asarray(pred), np.asarray(target), np.asarray(match))
    res = run_bass_kernel_spmd(nc, in_maps, core_ids=list(range(M)))
    return _finish([r["out"] for r in res.results])


# revision 24
# speedup vs baseline: 1.0968x; 1.0968x over previous
"""Associative-embedding loss kernel for 8 Trainium2 NeuronCores.

Math: per image b, with tl[n,c] = pred[b,c,ty,tx] and br[n,c] = target[b,c,by,bx]
gathered at the N=128 match points:
  pull_b = sum_{n,c} (tl-br)^2 / (2N)
  s[n]   = 0.5 * sum_c (tl+br),  A[i,j] = s[i]-s[j]
  push_b = sum_{i!=j} relu(1-|A[i,j]|) / (N(N-1))
Using antisymmetry of A:  sum_{ij} relu(1-|A|) = sum|A+1| - sum|A|, and the
diagonal contributes exactly N, so push_b = (sum|A+1| - sum|A| - N)/(N(N-1)).

Strategy: data-parallel over B (8 images per core). Inputs are relaid out
channels-last on the host so each match point is one contiguous 16B gather.
The device kernel issues 16 indirect DMAs (one per image x corner, 128
descriptors each); each image's compute (channel sums, two tiny PE
transposes, one K=2 matmul building the pairwise-difference matrix, and
|A+1|/|A| accumulating reductions) is hidden under the next image's
descriptor generation. Each core returns [128, 24] partial sums that the
host folds into the two scalars.
"""

import numpy as np

B, C, H, W, N = 64, 4, 256, 256, 128
M = 8            # cores
BL = B // M      # images per core
HW = H * W

_GRAPH = None


def _build_graph():
    import concourse.bass as bass
    import concourse.bacc as bacc
    import concourse.mybir as mybir
    from concourse.tile import TileContext

    f32 = mybir.dt.float32
    i32 = mybir.dt.int32
    Alu = mybir.AluOpType
    Act = mybir.ActivationFunctionType

    nc = bacc.Bacc()
    pt_d = nc.declare_dram_parameter("pt", [1024, 1024, C], f32, isOutput=False)
    idx_d = nc.declare_dram_parameter("idx", [N, 2 * BL], i32, isOutput=False)
    ident_d = nc.declare_dram_parameter("ident", [128, 128], f32, isOutput=False)
    out_d = nc.declare_dram_parameter("out", [N, 3 * BL], f32, isOutput=True)

    with TileContext(nc) as tc:
        with (
            tc.tile_pool(name="sb", bufs=1) as pool,
            tc.tile_pool(name="w", bufs=1) as wpool,
            tc.tile_pool(name="ps", bufs=2, space="PSUM") as psum,
        ):
            idx_t = pool.tile([N, 2 * BL], i32)
            nc.sync.dma_start(out=idx_t[:], in_=idx_d[:])
            ident = pool.tile([128, 128], f32)
            nc.sync.dma_start(out=ident[:], in_=ident_d[:])

            acc = pool.tile([N, 3 * BL], f32)

            # pre-fill the transpose sources with ones (off the critical path)
            x2a = [wpool.tile([N, 2], f32, name=f"x2a{b}", tag=f"x2a{b}") for b in range(BL)]
            x2b = [wpool.tile([N, 2], f32, name=f"x2b{b}", tag=f"x2b{b}") for b in range(BL)]
            for b in range(BL):
                nc.vector.memset(x2a[b][:], 1.0)
                nc.vector.memset(x2b[b][:], 1.0)

            # idx col 2b = tl of image b, col 2b+1 = br of image b
            g = [wpool.tile([N, 2 * C], f32, name=f"g{b}", tag=f"g{b}") for b in range(BL)]
            for b in range(BL):
                for half in range(2):
                    k = 2 * b + half
                    nc.gpsimd.indirect_dma_start(
                        out=g[b][:, C * half:C * (half + 1)], out_offset=None,
                        in_=pt_d[:],
                        in_offset=bass.IndirectOffsetOnAxis(
                            ap=idx_t[:, k:k + 1], axis=1),
                    )

            def image_front(b):
                tl = g[b][:, 0:C]
                br = g[b][:, C:2 * C]

                # pull: acc[:, 3b] = sum_c (tl-br)^2
                d = wpool.tile([N, C], f32, tag=f"d{b}")
                nc.vector.tensor_sub(d[:], tl, br)
                d2 = wpool.tile([N, C], f32, tag=f"d2{b}")
                nc.scalar.activation(
                    out=d2[:], in_=d[:], func=Act.Square,
                    accum_out=acc[:, 3 * b:3 * b + 1],
                )

                # t = sum_c (tl+br) = 2*s
                e = wpool.tile([N, C], f32, tag=f"e{b}")
                nc.vector.tensor_add(e[:], tl, br)
                t = wpool.tile([N, 1], f32, tag=f"t{b}")
                nc.vector.tensor_reduce(
                    out=t[:], in_=e[:], axis=mybir.AxisListType.X, op=Alu.add)

                # x2a cols = [0.5t, 1]; x2b cols = [1, -0.5t]
                nc.scalar.activation(out=x2a[b][:, 0:1], in_=t[:], func=Act.Copy, scale=0.5)
                nc.scalar.activation(out=x2b[b][:, 1:2], in_=t[:], func=Act.Copy, scale=-0.5)

                ta_ps = psum.tile([2, 128], f32, tag="ta")
                tb_ps = psum.tile([2, 128], f32, tag="tb")
                nc.tensor.transpose(out=ta_ps[:], in_=x2a[b][:], identity=ident[:])
                nc.tensor.transpose(out=tb_ps[:], in_=x2b[b][:], identity=ident[:])
                lt = wpool.tile([2, 128], f32, tag=f"lt{b}")
                rs = wpool.tile([2, 128], f32, tag=f"rs{b}")
                nc.vector.tensor_copy(lt[:], ta_ps[:])
                nc.vector.tensor_copy(rs[:], tb_ps[:])

                # A[i,j] = 0.5t[i] - 0.5t[j]
                a_ps = psum.tile([128, 128], f32, tag="a", bufs=4)
                nc.tensor.matmul(out=a_ps[:], lhsT=lt[:], rhs=rs[:],
                                 start=True, stop=True)

                # acc[:, 3b+1] = sum_j |A+1|, acc[:, 3b+2] = sum_j |A|
                scr = wpool.tile([128, 128], f32, tag=f"scr{b}")
                nc.scalar.activation(
                    out=scr[:], in_=a_ps[:], func=Act.Abs, bias=1.0, scale=1.0,
                    accum_out=acc[:, 3 * b + 1:3 * b + 2],
                )
                nc.vector.tensor_reduce(
                    out=acc[:, 3 * b + 2:3 * b + 3], in_=a_ps[:],
                    axis=mybir.AxisListType.X, op=Alu.add,
                    apply_absolute_value=True,
                )

            for b in range(BL - 2):
                image_back(b, *image_front(b))
            fr6 = image_front(BL - 2)
            fr7 = image_front(BL - 1)
            image_back(BL - 2, *fr6)
            image_back(BL - 1, *fr7)

            nc.sync.dma_start(out=out_d[:, 0:3 * (BL - 2)], in_=acc[:, 0:3 * (BL - 2)])
            nc.sync.dma_start(out=out_d[:, 3 * (BL - 2):], in_=acc[:, 3 * (BL - 2):])
    nc.finalize()
    return nc


def _get_graph():
    global _GRAPH
    if _GRAPH is None:
        _GRAPH = _build_graph()
    return _GRAPH


def _make_in_maps(pred, target, match):
    pred_cl = np.ascontiguousarray(np.transpose(pred, (0, 2, 3, 1)))
    targ_cl = np.ascontiguousarray(np.transpose(target, (0, 2, 3, 1)))
    ident = np.eye(128, dtype=np.float32)
    in_maps = []
    base = (np.arange(BL, dtype=np.int64) * HW)[:, None]
    for i in range(M):
        sl = slice(i * BL, (i + 1) * BL)
        m = match[sl].astype(np.int64)
        itl = base + m[:, :, 0, 0] * W + m[:, :, 0, 1]            # [BL, N]
        ibr = BL * HW + base + m[:, :, 1, 0] * W + m[:, :, 1, 1]  # [BL, N]
        # interleave: col 2b = tl_b, col 2b+1 = br_b
        idx = np.empty((N, 2 * BL), np.int32)
        idx[:, 0::2] = itl.T
        idx[:, 1::2] = ibr.T
        pt = np.concatenate(
            [pred_cl[sl].reshape(512, 1024, C), targ_cl[sl].reshape(512, 1024, C)],
            axis=0,
        )
        in_maps.append({"pt": pt, "idx": idx, "ident": ident})
    return in_maps


def _finish(core_outs):
    pull_total = 0.0
    m_total = 0.0
    for o in core_outs:
        o = o.astype(np.float64).reshape(N, BL, 3)
        pull_total += o[:, :, 0].sum()
        m_total += (o[:, :, 1] - o[:, :, 2]).sum()
    pull_all = 0.25 * pull_total / (2 * N)
    push_all = 0.25 * (m_total - B * N) / (N * (N - 1))
    return (np.float32(pull_all), np.float32(push_all))


def kernel(pred, target, match):
    from concourse.bass_utils import run_bass_kernel_spmd

    nc = _get_graph()
    in_maps = _make_in_maps(np.asarray(pred), np.asarray(target), np.asarray(match))
    res = run_bass_kernel_spmd(nc, in_maps, core_ids=list(range(M)))
    return _finish([r["out"] for r in res.results])
